# revision 3
# baseline (speedup 1.0000x reference)
"""CoAttention kernel for Trainium2, 8 NeuronCores, data-parallel over batch.

Reference computation (per batch b):
    k_proj = key @ W_k.T + b_k            # (S, D)
    scores = query @ k_proj.T             # (S, S)
    scores += log(cell_mask) + log(seq_mask)[None, :]
    p = softmax(scores, axis=-1)
    out = (p @ value) @ W_o.T + b_o       # (S, D)

Algebraic simplifications used (exact):
  - scores = query @ W_k @ key.T + (query @ b_k)[:, None]; the b_k term is
    constant along the softmax axis, so softmax is invariant to it -> b_k
    is dropped entirely.
  - cell_mask/seq_mask are all-ones per the problem spec (log == 0); the
    kernel checks this on the host and falls back to a numpy path if not.
  - b_o is added on the host (it is all-zeros per spec, but handled exactly).
  - softmax normalization (1/rowsum) is folded into the final PSUM
    evacuation of the output matmul, so the attention tail runs on
    unnormalized exp weights.

Precision scheme:
  - scores path (q_projT = W_k^T @ queryT, scores = q_projT^T @ keyT) runs
    entirely in fp32r: full fp32 operands through the PE at fp16 rate for
    moving dims >= 256 (1 cycle/row), single pass. Transposes in fp32r cost
    1.5 cycles/row.
  - softmax: fp32 row max (negated) on DVE, exp on ScalarE with fused
    fp32 row-sum; exp output stored f16 (unnormalized).
  - tail (p @ value, x @ W_o.T): fp16 operands, fp32 PSUM accumulation,
    1/rowsum applied on the final DVE evacuation.

Schedule: phase 0a interleaves the q_projT matmul chunks with query
tile load/transpose so PE never starves; phase 0b interleaves the scores
matmuls of the first q-block with the keyT build; the main loop is
software-pipelined as head-matmuls(qb) / tail(qb-1) / softmax(qb) with the
last scores chunk split into two 4-matmul filler batches that plug PE
bubbles inside the tail.
"""

import numpy as np

import concourse.bass as bass
import concourse.mybir as mybir
import concourse.tile as tile
from concourse import bacc
from concourse.bass_utils import run_bass_kernel_spmd
from concourse.masks import make_identity

P = 128
S = 2048
D = 1024
NBS = S // P   # 16 row blocks of seq
NBD = D // P   # 8 row blocks of feature dim
NC = 8         # cores == batch
F32 = mybir.dt.float32
F16 = mybir.dt.float16
F32R = mybir.dt.float32r


def build_nc(repeat=1):
    nc = bacc.Bacc("TRN2", target_bir_lowering=False, debug=False)
    # f32r is bit-identical to f32; declaring the DRAM side f32r lets the
    # loads land directly in f32r SBUF tiles with no conversion pass.
    d_query = nc.dram_tensor("query", [S, D], F32R, kind="ExternalInput")
    d_key = nc.dram_tensor("key", [S, D], F32R, kind="ExternalInput")
    d_value = nc.dram_tensor("value", [S, D], F32, kind="ExternalInput")
    d_wk = nc.dram_tensor("W_k", [D, D], F32R, kind="ExternalInput")
    d_wo = nc.dram_tensor("W_o", [D, D], F32R, kind="ExternalInput")
    d_out = nc.dram_tensor("out", [S, D], F32, kind="ExternalOutput")

    with tile.TileContext(nc) as tc:
      def emit_body():
            # ---------------- constants ----------------
            const_pool = tc.alloc_tile_pool(name="const", bufs=1)
            ident16 = const_pool.tile([P, P], F16)
            make_identity(nc, ident16[:])
            ident32 = const_pool.tile([P, P], F32)
            make_identity(nc, ident32[:])
            ident_r = const_pool.tile([P, P], F32R)
            nc.vector.tensor_copy(ident_r[:], ident32[:])

            def tr8(ps_pool, dst3d, src2d, qi, eng_i=0):
                """Transpose NBD 128x128 f32r blocks of src2d [P, D] into
                column qi of dst3d [P, NBD, cols] via one PSUM buffer and one
                strided copy (engine alternates by eng_i)."""
                t = ps_pool.tile([P, NBD * P], F32R, tag="tp", name=f"tp_{qi}")
                for j in range(NBD):
                    nc.tensor.transpose(t[:, j * P:(j + 1) * P],
                                        src2d[:, j * P:(j + 1) * P], ident_r[:])
                dst = dst3d[:, :, qi * P:(qi + 1) * P]
                src = t[:].rearrange("p (j q) -> p j q", j=NBD)
                if eng_i % 2 == 0:
                    nc.scalar.copy(dst, src)
                else:
                    nc.vector.tensor_copy(dst, src)

            # ---------------- resident: q_projT ----------------
            # qpT: [P, NBD*S] f32r; feature block db at columns [db*S, (db+1)*S)
            qpT_pool = tc.alloc_tile_pool(name="qpT", bufs=1)
            qpT = qpT_pool.tile([P, NBD * S], F32R, name="qpT")

            # ============ phase 0a: query transpose + W_k + q_projT ========
            with tc.tile_pool(name="p0a_sb", bufs=3) as p0a_sb, \
                 tc.tile_pool(name="p0a_qt", bufs=1) as p0a_qt, \
                 tc.tile_pool(name="p0a_wk", bufs=1) as p0a_wk, \
                 tc.tile_pool(name="p0a_ps", bufs=2, space="PSUM") as p0a_ps, \
                 tc.tile_pool(name="p0a_ps2", bufs=4, space="PSUM") as p0a_ps2:

                qt = p0a_qt.tile([P, NBD * S], F32R, name="qt")
                qt3 = qt[:].rearrange("p (j s) -> p j s", j=NBD)
                wk = [p0a_wk.tile([P, D], F32R, name=f"wk{i}")
                      for i in range(NBD)]

                def do_query_tile(qi):
                    q_ld = p0a_sb.tile([P, D], F32R, tag="ld", name=f"qld{qi}")
                    nc.sync.dma_start(q_ld[:], d_query[qi * P:(qi + 1) * P, :])
                    tr8(p0a_ps, qt3, q_ld[:], qi, eng_i=qi)

                def do_qp_chunk(qc):
                    # q_projT[d, qc-cols] = sum_{d'} W_k[d', d] * QT[d', cols]
                    for db in range(NBD):
                        ps = p0a_ps2.tile([P, 512], F32, tag="qp",
                                          name=f"qp{db}_{qc}")
                        for dpb in range(NBD):
                            nc.tensor.matmul(
                                ps[:],
                                wk[dpb][:, db * P:(db + 1) * P],
                                qt[:, dpb * S + qc * 512:
                                   dpb * S + (qc + 1) * 512],
                                start=(dpb == 0), stop=(dpb == NBD - 1))
                        off = db * S + qc * 512
                        if db % 2 == 0:
                            nc.scalar.copy(qpT[:, off:off + 512], ps[:])
                        else:
                            nc.vector.tensor_copy(qpT[:, off:off + 512], ps[:])

                for qi in range(4):
                    do_query_tile(qi)
                for i in range(NBD):
                    wk_ld = p0a_sb.tile([P, D], F32R, tag="ld", name=f"wkld{i}")
                    nc.sync.dma_start(wk_ld[:], d_wk[i * P:(i + 1) * P, :])
                    nc.gpsimd.tensor_copy(wk[i][:], wk_ld[:])
                for qc in range(4):
                    do_qp_chunk(qc)
                    if qc < 3:
                        for qi in range(4 * (qc + 1), 4 * (qc + 2)):
                            do_query_tile(qi)

            # scores PSUM lives from phase 0b (first q-block overlap) onward
            sc_ps = tc.alloc_tile_pool(name="sc_ps", bufs=1, space="PSUM")

            # ---------------- resident: keyT, value, W_oT ----------------
            kT_pool = tc.alloc_tile_pool(name="kT", bufs=1)
            v_pool = tc.alloc_tile_pool(name="v", bufs=1)
            wo_pool = tc.alloc_tile_pool(name="wo", bufs=1)
            kT = kT_pool.tile([P, NBD * S], F32R, name="kT")
            kT3 = kT[:].rearrange("p (j s) -> p j s", j=NBD)
            vv = [v_pool.tile([P, D], F16, name=f"v_{i}") for i in range(NBS)]
            woT = wo_pool.tile([P, NBD * D], F16, name="woT")
            woT3 = woT[:].rearrange("p (j o) -> p j o", j=NBD)

            # softmax-state pools (used from phase 0b for q-block 0)
            exp_sb = tc.alloc_tile_pool(name="exp_sb", bufs=2)
            st_sb = tc.alloc_tile_pool(name="st_sb", bufs=2)

            state = {}

            def head_mm_chunk(qb, kc, scores, half=None):
                """Emit the scores matmuls for 512-col chunk kc of q-block qb.

                half=None emits all NBD accumulating matmuls; half=0/1 emits
                only the first/second 4 (same accumulation group, split for
                use as PE filler inside tail())."""
                q0 = qb * P
                rng = range(NBD) if half is None else \
                    range(half * 4, (half + 1) * 4)
                for db in rng:
                    nc.tensor.matmul(
                        scores[:, kc * 512:(kc + 1) * 512],
                        qpT[:, db * S + q0:db * S + q0 + P],
                        kT[:, db * S + kc * 512:db * S + (kc + 1) * 512],
                        start=(db == 0), stop=(db == NBD - 1))

            def head_mm(qb, chunks=None):
                if qb not in state:
                    state[qb] = {"scores": sc_ps.tile([P, S], F32, tag="scores",
                                                      name=f"scores{qb}")}
                scores = state[qb]["scores"]
                for kc in (range(S // 512) if chunks is None else chunks):
                    head_mm_chunk(qb, kc, scores)

            def head_softmax(qb):
                st = state[qb]
                scores = st["scores"]
                neg_max = st_sb.tile([P, 1], F32, tag="negmax", name=f"nm{qb}")
                nc.vector.reduce_max(neg_max[:], scores[:],
                                     axis=mybir.AxisListType.X, negate=True)
                rowsum = st_sb.tile([P, 1], F32, tag="rowsum", name=f"rs{qb}")
                expv = exp_sb.tile([P, S], F16, tag="expv", name=f"expv{qb}")
                nc.scalar.activation(expv[:], scores[:],
                                     mybir.ActivationFunctionType.Exp,
                                     bias=neg_max[:], scale=1.0,
                                     accum_out=rowsum[:])
                recip = st_sb.tile([P, 1], F32, tag="recip", name=f"recip{qb}")
                nc.vector.reciprocal(recip[:], rowsum[:])
                st["expv"] = expv
                st["recip"] = recip

            # ============ phase 0b: keyT/value/W_o build + scores(0) =======
            with tc.tile_pool(name="p0b_sb", bufs=2) as p0b_sb, \
                 tc.tile_pool(name="p0b_ps", bufs=2, space="PSUM") as p0b_ps:

                scores0 = sc_ps.tile([P, S], F32, tag="scores", name="scores_0")
                state[0] = {"scores": scores0}

                def do_wo_tile(oi):
                    wo_ld = p0b_sb.tile([P, D], F32R, tag="ld", name=f"wold{oi}")
                    nc.sync.dma_start(wo_ld[:], d_wo[oi * P:(oi + 1) * P, :])
                    t = p0b_ps.tile([P, NBD * P], F32R, tag="tp",
                                    name=f"twoT{oi}")
                    for j in range(NBD):
                        nc.tensor.transpose(t[:, j * P:(j + 1) * P],
                                            wo_ld[:, j * P:(j + 1) * P],
                                            ident_r[:])
                    nc.scalar.copy(woT3[:, :, oi * P:(oi + 1) * P],
                                   t[:].rearrange("p (j q) -> p j q", j=NBD))

                for kc in range(4):
                    for ki in range(4 * kc, 4 * (kc + 1)):
                        k_ld = p0b_sb.tile([P, D], F32R, tag="ld",
                                           name=f"kld{ki}")
                        nc.sync.dma_start(k_ld[:], d_key[ki * P:(ki + 1) * P, :])
                        tr8(p0b_ps, kT3, k_ld[:], ki, eng_i=ki)
                    head_mm_chunk(0, kc, scores0)

                head_softmax(0)

                for ki in range(NBS):
                    v_ld = p0b_sb.tile([P, D], F32, tag="ldv", name=f"vld{ki}")
                    nc.sync.dma_start(v_ld[:], d_value[ki * P:(ki + 1) * P, :])
                    nc.gpsimd.tensor_copy(vv[ki][:], v_ld[:])
                    if ki % 2 == 0:
                        do_wo_tile(ki // 2)

            # ============ main loop over q blocks (software-pipelined) =====
            tr_ps = tc.alloc_tile_pool(name="tr_ps", bufs=2, space="PSUM")
            xo_ps = tc.alloc_tile_pool(name="xo_ps", bufs=2, space="PSUM")
            pt_sb = tc.alloc_tile_pool(name="pt_sb", bufs=3)
            xn_sb = tc.alloc_tile_pool(name="xn_sb", bufs=2)
            xt_sb = tc.alloc_tile_pool(name="xt_sb", bufs=2)
            out_sb = tc.alloc_tile_pool(name="out_sb", bufs=2)

            def tail(qb, filler=()):
                """Emit the post-softmax pipeline for q-block qb. `filler` is
                a list of callables emitting small PE matmul batches (halves
                of the next block's last scores chunk) used to keep PE busy
                while ACT/DVE evacuate transpose banks."""
                filler = list(filler)

                def fill():
                    if filler:
                        filler.pop(0)()

                st = state.pop(qb)
                expv, recip = st["expv"], st["recip"]
                # transpose unnormalized P into pT, 8 blocks per PSUM buffer
                pts = []
                for g in range(2):
                    ptp = tr_ps.tile([P, 8 * P], F16, tag="trp",
                                     name=f"ptp{qb}_{g}")
                    for j in range(8):
                        kb = g * 8 + j
                        nc.tensor.transpose(
                            ptp[:, j * P:(j + 1) * P],
                            expv[:, kb * P:(kb + 1) * P], ident16[:])
                    pt = pt_sb.tile([P, 8 * P], F16, tag="pt", name=f"pt{qb}_{g}")
                    nc.scalar.copy(pt[:], ptp[:])
                    pts.append(pt)
                    if g == 0:
                        fill()

                # x = expv @ V (unnormalized), d in halves
                xn = xn_sb.tile([P, D], F16, tag="xn", name=f"xn{qb}")
                for dh in range(2):
                    xp = xo_ps.tile([P, 512], F32, tag="xo", name=f"xp{qb}_{dh}")
                    for kb in range(NBS):
                        nc.tensor.matmul(
                            xp[:],
                            pts[kb // 8][:, (kb % 8) * P:(kb % 8 + 1) * P],
                            vv[kb][:, dh * 512:(dh + 1) * 512],
                            start=(kb == 0), stop=(kb == NBS - 1))
                    nc.vector.tensor_copy(xn[:, dh * 512:(dh + 1) * 512], xp[:])

                fill()
                # transpose xn -> xT (one PSUM buffer, 8 blocks); copy on ACT
                xtp = tr_ps.tile([P, 8 * P], F16, tag="trp", name=f"xtp{qb}")
                for j in range(NBD):
                    nc.tensor.transpose(xtp[:, j * P:(j + 1) * P],
                                        xn[:, j * P:(j + 1) * P], ident16[:])
                xt = xt_sb.tile([P, 8 * P], F16, tag="xt", name=f"xt{qb}")
                nc.scalar.copy(xt[:], xtp[:])
                while filler:
                    filler.pop(0)()

                # out = xT.T @ woT (unnormalized), o in halves; recip folds
                # into the PSUM evacuation
                ops = [xo_ps.tile([P, 512], F32, tag="xo", name=f"op{qb}_{i}")
                       for i in range(2)]
                for db in range(NBD):
                    lhs = xt[:, db * P:(db + 1) * P]
                    for oh in range(2):
                        nc.tensor.matmul(
                            ops[oh][:], lhs,
                            woT[:, db * D + oh * 512:db * D + (oh + 1) * 512],
                            start=(db == 0), stop=(db == NBD - 1))
                q0 = qb * P
                for oh in range(2):
                    osb = out_sb.tile([P, 512], F32, tag="osb",
                                      name=f"osb{qb}_{oh}")
                    nc.vector.tensor_scalar_mul(osb[:], ops[oh][:], recip[:])
                    nc.sync.dma_start(
                        d_out[q0:q0 + P, oh * 512:(oh + 1) * 512], osb[:])

            last_kc = S // 512 - 1
            for qb in range(1, NBS + 1):
                filler = []
                if qb < NBS:
                    head_mm(qb, chunks=range(last_kc))
                    sc_n = state[qb]["scores"]
                    filler = [
                        (lambda h=hi, q=qb, s=sc_n:
                         head_mm_chunk(q, last_kc, s, half=h))
                        for hi in range(2)
                    ]
                tail(qb - 1, filler)
                if qb < NBS:
                    head_softmax(qb)

            out_sb.release()
            xt_sb.release()
            xn_sb.release()
            pt_sb.release()
            xo_ps.release()
            tr_ps.release()
            st_sb.release()
            exp_sb.release()
            wo_pool.release()
            v_pool.release()
            kT_pool.release()
            sc_ps.release()
            qpT_pool.release()
            const_pool.release()


      for _rep in range(repeat):
          emit_body()

    nc.compile()
    return nc


_NC_CACHE = {}


def _get_nc():
    if "nc" not in _NC_CACHE:
        _NC_CACHE["nc"] = build_nc()
    return _NC_CACHE["nc"]


def _numpy_fallback(query, key, value, cell_mask, seq_mask, W_k, b_k, W_o, b_o):
    out = np.empty((query.shape[0], S, D), dtype=np.float32)
    for b in range(query.shape[0]):
        kp = key[b].astype(np.float64) @ W_k.astype(np.float64).T + b_k
        s = query[b].astype(np.float64) @ kp.T
        s = s + np.log(cell_mask[b]) + np.log(seq_mask[b])[None, :]
        s -= s.max(1, keepdims=True)
        e = np.exp(s)
        p = e / e.sum(1, keepdims=True)
        x = p @ value[b].astype(np.float64)
        out[b] = (x @ W_o.astype(np.float64).T + b_o).astype(np.float32)
    return out


def kernel(query, key, value, cell_mask, seq_mask, W_k, b_k, W_o, b_o):
    query = np.ascontiguousarray(query, dtype=np.float32)
    key = np.ascontiguousarray(key, dtype=np.float32)
    value = np.ascontiguousarray(value, dtype=np.float32)
    W_k = np.ascontiguousarray(W_k, dtype=np.float32)
    W_o = np.ascontiguousarray(W_o, dtype=np.float32)

    # masks are all-ones per the problem spec -> log-mask bias is exactly 0.
    # b_k shifts every score row by a constant -> softmax-invariant (exact).
    if not (np.all(np.asarray(cell_mask) == 1.0)
            and np.all(np.asarray(seq_mask) == 1.0)):
        return _numpy_fallback(np.asarray(query), np.asarray(key),
                               np.asarray(value), np.asarray(cell_mask),
                               np.asarray(seq_mask), W_k,
                               np.asarray(b_k), W_o, np.asarray(b_o))

    nc = _get_nc()
    in_maps = [
        {"query": query[b], "key": key[b], "value": value[b],
         "W_k": W_k, "W_o": W_o}
        for b in range(NC)
    ]
    res = run_bass_kernel_spmd(nc, in_maps, core_ids=list(range(NC)))
    out = np.stack([res.results[b]["out"] for b in range(NC)])
    if b_o is not None and np.any(np.asarray(b_o) != 0.0):
        out = out + np.asarray(b_o, dtype=np.float32)[None, None, :]
    return out
